# revision 7
# baseline (speedup 1.0000x reference)
"""Trainium2 Bass kernel for 3-layer GAT + global_add_pool + linear head.

Sharding: nodes (and incoming edges) partitioned across 8 cores by dst; the
per-layer node table (rows [a_d | a_s | h], bf16, 256-elem rows) is exchanged
with an AllGather; per-edge source rows are fetched with chunked
InstDMAGatherAnt gathers (up to 32 tiles = 4096 rows per instruction, int16
bank-relative indices over two <32768-row table banks); per-edge a_d is
fetched from the local table with a second chunked gather (elem_step trick:
256B half-rows by local dst id). The segment softmax + weighted aggregation
run as bf16 selector-matrix matmuls on the PE with fp32 PSUM accumulation per
128-node dst block. LeakyReLU runs on the vector engine ((x*0.2) max x), so
Exp is the only scalar-engine table function. Partial pooled logits are
summed on the host (the final all-reduce of [64,10] x 8).

kernel() is self-contained: shapes hardcoded, no file reads.
"""
import math
import numpy as np
from contextlib import ExitStack

import ml_dtypes

import concourse.bass as bass
import concourse.mybir as mybir
import concourse.tile as tile
from concourse.bass_utils import run_bass_kernel_spmd
from concourse.library_config import mlp
from concourse.tile_rust import add_dep_helper
from concourse.masks import make_identity

NCORES = 8
P = 128
H = 4
Ch = 32
HC = 128            # H * Ch
AUG = HC + 2 * H    # 136: [a_d | a_s | h] row payload
ROW = 256           # bf16 table row elems (512B, dma_gather granule)
MSGW = HC + H       # 132: [msg | ex]
NEG_SLOPE = 0.2
GRAPHS = 64
OUT = 10
CHUNK_TILES = 32    # max tiles (of 128 edges) per dma_gather
GROUP_BLOCKS = 2    # dst blocks per bank-interleave group (PSUM liveness)

BF16 = ml_dtypes.bfloat16

# instruction types whose BIR struct cannot carry all Tile-emitted waits
_WAIT_CAPS = {
    "InstDMAGatherAnt": 0,
    "InstDMAScatterAddAnt": 0,
    "InstNoOp": 1,
    "InstDrain": 1,
    "InstCollectiveCompute": 1,
}


def _fixup_wait_limits(nc):
    k = 0
    for fn in nc.m.functions:
        for blk in fn.blocks:
            out = []
            for inst in blk.instructions:
                cap = _WAIT_CAPS.get(type(inst).__name__, 1)
                si = inst.sync_info
                if si is not None:
                    waits = list(si.on_wait)
                    if len(waits) > cap:
                        keep, move = waits[:cap], waits[cap:]
                        for w in move:
                            nop = mybir.InstNoOp(name=f"waitfix_{k}", text_hint="wait_fixup")
                            k += 1
                            nop.engine = inst.engine
                            nop.sync_info = type(si)(on_wait=[w], on_update=[])
                            out.append(nop)
                        inst.sync_info = type(si)(on_wait=list(keep), on_update=list(si.on_update))
                out.append(inst)
            blk.instructions = out
    return k


class EdgePlan:
    """Uniform (across cores) tile/chunk structure + per-core index arrays."""
    __slots__ = ("tiles", "chunks", "Ttot", "srcw", "adw", "dlocs")


def _prep_edges(src_all, dst_all, per, nb, bank):
    """Tile/chunk planning.

    src_all/dst_all: all edges (incl self loops), global node ids; dst defines
    the owning core (dst // per). bank: global src bank boundary.

    Tiles are bank-pure (each tile's 128 edges all have src in one bank).
    Blocks are processed in groups of GROUP_BLOCKS; within a group all bank0
    tiles come first (chunked <=CHUNK_TILES per dma_gather), then bank1.
    Chunks are padded to a multiple of 4 tiles with dummy tiles.
    """
    core = dst_all // per
    loc = dst_all % per
    blk = loc // P

    # per (core, block): edges sorted by src, split by bank
    per_core = []   # [core][block] -> (src_sorted, loc_sorted)
    nt = np.zeros((nb, 2), np.int64)   # uniform tiles per (block, bank)
    for c in range(NCORES):
        m = core == c
        s, d, b = src_all[m], loc[m], blk[m]
        blocks = []
        for bb in range(nb):
            mb = b == bb
            sb_, db_ = s[mb], d[mb]
            order = np.argsort(sb_, kind="stable")
            sb_, db_ = sb_[order], db_[order]
            n0 = int((sb_ < bank).sum())
            blocks.append(((sb_[:n0], db_[:n0]), (sb_[n0:], db_[n0:])))
            nt[bb, 0] = max(nt[bb, 0], (n0 + P - 1) // P)
            nt[bb, 1] = max(nt[bb, 1], (len(sb_) - n0 + P - 1) // P)
        per_core.append(blocks)

    # uniform tile/chunk order
    tiles = []    # (block, bank, k) ; block=-1 for dummy pad tiles
    chunks = []   # (bank, t0, ntiles)
    for g0 in range(0, nb, GROUP_BLOCKS):
        gblocks = range(g0, min(g0 + GROUP_BLOCKS, nb))
        for bk in (0, 1):
            run = [(bb, bk, k) for bb in gblocks for k in range(int(nt[bb, bk]))]
            for i0 in range(0, len(run), CHUNK_TILES):
                part = run[i0:i0 + CHUNK_TILES]
                t0 = len(tiles)
                tiles.extend(part)
                while (len(tiles) - t0) % 4 != 0:
                    tiles.append((-1, bk, 0))
                chunks.append((bk, t0, len(tiles) - t0))
    Ttot = len(tiles)

    # start/stop flags: first/last REAL tile of each block in tile order
    first_t = {}
    last_t = {}
    for t, (bb, bk, k) in enumerate(tiles):
        if bb < 0:
            continue
        if bb not in first_t:
            first_t[bb] = t
        last_t[bb] = t
    tile_meta = []
    for t, (bb, bk, k) in enumerate(tiles):
        tile_meta.append((bb, bb >= 0 and first_t[bb] == t,
                          bb >= 0 and last_t[bb] == t))

    # per-core index arrays
    plan = EdgePlan()
    plan.tiles = tile_meta
    plan.chunks = chunks
    plan.Ttot = Ttot
    plan.srcw = []
    plan.adw = []
    plan.dlocs = []
    for c in range(NCORES):
        srcrel = np.zeros((P, Ttot), np.int64)
        adloc = np.zeros((P, Ttot), np.int64)
        dlocf = np.full((P, Ttot), -1.0, np.float32)
        for t, (bb, bk, k) in enumerate(tiles):
            if bb < 0:
                continue
            s, d = per_core[c][bb][bk]
            seg_s = s[k * P:(k + 1) * P]
            seg_d = d[k * P:(k + 1) * P]
            n = len(seg_s)
            if n:
                srcrel[:n, t] = seg_s - (bank if bk else 0)
                adloc[:n, t] = seg_d
                dlocf[:n, t] = (seg_d % P).astype(np.float32)
        # int16 wrap: index i (tile j slot s; i = j*128+s) at [i%16, i//16]
        def wrap(arr):
            w = np.zeros((16, Ttot * 8), np.int16)
            i = np.arange(Ttot * P)
            j, sl = i // P, i % P
            w[i % 16, i // 16] = arr[sl, j].astype(np.int16)
            return np.tile(w, (8, 1))
        plan.srcw.append(wrap(srcrel))
        plan.adw.append(wrap(adloc))
        plan.dlocs.append(dlocf.astype(BF16))
    return plan


def _build(npad, bank, plan):
    per = npad // NCORES
    nb = per // P
    nlayers = 3
    f32 = mybir.dt.float32
    bf16 = mybir.dt.bfloat16
    i16 = mybir.dt.int16
    Ttot = plan.Ttot

    nc = bass.Bass(num_devices=NCORES, num_swdge_queues=2)
    # ---- dram I/O
    xT_d = nc.dram_tensor("xT", [P, per], f32, kind="ExternalInput")
    waug_d = nc.dram_tensor("waug", [nlayers, P, AUG], f32, kind="ExternalInput")
    wh_d = nc.dram_tensor("wh", [P, OUT], f32, kind="ExternalInput")
    iota_d = nc.dram_tensor("iota", [P, 4 * P], bf16, kind="ExternalInput")
    srci_d = nc.dram_tensor("srci", [P, Ttot * 8], i16, kind="ExternalInput")
    adi_d = nc.dram_tensor("adi", [P, Ttot * 8], i16, kind="ExternalInput")
    dloc_d = nc.dram_tensor("dloc", [P, Ttot], bf16, kind="ExternalInput")
    batchf_d = nc.dram_tensor("batchf", [P, nb], bf16, kind="ExternalInput")
    out_d = nc.dram_tensor("out", [GRAPHS, OUT], f32, kind="ExternalOutput")

    h_loc = [nc.dram_tensor(f"h_loc{l}", [per, ROW], bf16) for l in range(nlayers)]
    h_full = [nc.dram_tensor(f"h_full{l}", [npad, ROW], bf16, addr_space="Shared")
              for l in range(nlayers)]

    groups = [list(range(NCORES))]

    with ExitStack() as ctx:
        tc = ctx.enter_context(tile.TileContext(nc))
        sb = ctx.enter_context(tc.tile_pool(name="sb", bufs=1))
        sb_g = ctx.enter_context(tc.tile_pool(name="sbg", bufs=2))
        sb_w = ctx.enter_context(tc.tile_pool(name="sbw", bufs=3))
        ps_h = ctx.enter_context(tc.tile_pool(name="psh", bufs=1, space="PSUM"))
        ps_agg = ctx.enter_context(
            tc.tile_pool(name="psagg", bufs=GROUP_BLOCKS + 1, space="PSUM"))
        ps_xp = ctx.enter_context(tc.tile_pool(name="psxp", bufs=1, space="PSUM"))
        ps_fin = ctx.enter_context(tc.tile_pool(name="psfin", bufs=1, space="PSUM"))

        nc.gpsimd.load_library(mlp)

        # ---- persistent SBUF state
        xT = sb.tile([P, per], f32)
        nc.sync.dma_start(out=xT[:], in_=xT_d[:])
        waug = sb.tile([P, nlayers, AUG], f32)
        nc.sync.dma_start(out=waug[:], in_=waug_d[:].rearrange("l p a -> p l a"))
        wh = sb.tile([P, OUT], f32)
        nc.sync.dma_start(out=wh[:], in_=wh_d[:])
        iota = sb.tile([P, 4, P], bf16)
        nc.sync.dma_start(out=iota[:].rearrange("p a b -> p (a b)"), in_=iota_d[:])
        srci = sb.tile([P, Ttot * 8], i16)
        nc.sync.dma_start(out=srci[:], in_=srci_d[:])
        adi = sb.tile([P, Ttot * 8], i16)
        nc.sync.dma_start(out=adi[:], in_=adi_d[:])
        dloc = sb.tile([P, Ttot, 1], bf16)
        nc.sync.dma_start(out=dloc[:].rearrange("p t o -> p (t o)"), in_=dloc_d[:])
        batchf = sb.tile([P, nb, 1], bf16)
        nc.sync.dma_start(out=batchf[:].rearrange("p b o -> p (b o)"), in_=batchf_d[:])
        ident = sb.tile([P, P], f32)
        make_identity(nc, ident[:])

        hsb = sb.tile([P, nb, ROW], bf16)
        nc.vector.memset(hsb[:].rearrange("p b d -> p (b d)"), 0.0)
        pooled_ps = ps_fin.tile([GRAPHS, HC], f32)

        # one Pool register per distinct gather size (to_reg per call leaks)
        ni_regs = {}
        for (_bk, _t0, _ntc) in plan.chunks:
            ni = _ntc * P
            if ni not in ni_regs:
                ni_regs[ni] = nc.gpsimd.to_reg(ni)

        for l in range(3):
            # ===== node phase: h_aug = x @ W_aug; rows [a_d | a_s | h] =====
            for b in range(nb):
                ps = ps_h.tile([P, AUG], f32)
                nc.tensor.matmul(ps[:], lhsT=xT[:, b * P:(b + 1) * P],
                                 rhs=waug[:, l, :], start=True, stop=True)
                nc.vector.tensor_copy(out=hsb[:, b, 0:AUG], in_=ps[:])
            dh = nc.sync.dma_start(
                out=h_loc[l][:].rearrange("(b p) d -> p b d", p=P),
                in_=hsb[:])
            cch = nc.gpsimd.collective_compute(
                "AllGather", mybir.AluOpType.bypass, replica_groups=groups,
                ins=[h_loc[l][:]], outs=[h_full[l][:]])
            add_dep_helper(cch.ins, dh.ins, sync=True, reason="h write before ag")

            # ===== edge phase =====
            agg_of_blk = {}
            for (bk, t0, ntc) in plan.chunks:
                ni = ntc * P
                g = sb_g.tile([P, CHUNK_TILES, ROW], bf16, tag="gath")
                src_tab = h_full[l][bank:, :] if bk else h_full[l][0:bank, :]
                gi = nc.gpsimd.dma_gather(
                    out_ap=g[:, 0:ntc, :], in_ap=src_tab,
                    idxs_ap=srci[:, t0 * 8:(t0 + ntc) * 8],
                    num_idxs=ni, num_idxs_reg=ni_regs[ni], elem_size=ROW,
                    queue_num=0, single_packet=False)
                add_dep_helper(gi.ins, cch.ins, sync=True, reason="gather after ag")
                adg = sb_g.tile([P, CHUNK_TILES, P], bf16, tag="adg")
                ga = nc.gpsimd.dma_gather(
                    out_ap=adg[:, 0:ntc, :], in_ap=h_loc[l][:, 0:P],
                    idxs_ap=adi[:, t0 * 8:(t0 + ntc) * 8],
                    num_idxs=ni, num_idxs_reg=ni_regs[ni], elem_size=P,
                    elem_step=ROW, queue_num=1, single_packet=False)
                add_dep_helper(ga.ins, dh.ins, sync=True, reason="ad after h write")

                for q in range(ntc // 4):
                    tq = t0 + 4 * q
                    sel4 = sb_w.tile([P, 4, P], bf16, tag="sel")
                    nc.vector.tensor_tensor(
                        out=sel4[:],
                        in0=dloc[:, tq:tq + 4, :].to_broadcast([P, 4, P]),
                        in1=iota[:], op=mybir.AluOpType.is_equal)
                    lg4 = sb_w.tile([P, 4, H], f32, tag="lg")
                    nc.vector.tensor_tensor(
                        out=lg4[:], in0=g[:, 4 * q:4 * q + 4, H:2 * H],
                        in1=adg[:, 4 * q:4 * q + 4, 0:H],
                        op=mybir.AluOpType.add)
                    lr4 = sb_w.tile([P, 4, H], f32, tag="lr")
                    nc.vector.scalar_tensor_tensor(
                        out=lr4[:], in0=lg4[:], scalar=NEG_SLOPE, in1=lg4[:],
                        op0=mybir.AluOpType.mult, op1=mybir.AluOpType.max)
                    ex4 = sb_w.tile([P, 4, H, 1], bf16, tag="ex")
                    nc.scalar.activation(ex4[:].rearrange("p a b o -> p a (b o)"),
                                         lr4[:],
                                         mybir.ActivationFunctionType.Exp)
                    msg4 = sb_w.tile([P, 4, MSGW], bf16, tag="msg")
                    nc.vector.tensor_tensor(
                        out=msg4[:, :, 0:HC].rearrange("p q (h c) -> p q h c", h=H),
                        in0=g[:, 4 * q:4 * q + 4, 2 * H:2 * H + HC]
                            .rearrange("p q (h c) -> p q h c", h=H),
                        in1=ex4[:].to_broadcast([P, 4, H, Ch]),
                        op=mybir.AluOpType.mult)
                    nc.vector.tensor_copy(
                        out=msg4[:, :, HC:MSGW],
                        in_=ex4[:].rearrange("p a b o -> p a (b o)"))
                    for j in range(4):
                        t = tq + j
                        bb, st, sp = plan.tiles[t]
                        if bb < 0:
                            continue
                        if st:
                            agg_of_blk[bb] = ps_agg.tile(
                                [P, MSGW], f32, tag="agg", name=f"agg{l}_{bb}")
                        nc.tensor.matmul(agg_of_blk[bb][:], lhsT=sel4[:, j, :],
                                         rhs=msg4[:, j, :],
                                         start=bool(st), stop=bool(sp))
                        if sp:
                            agg = agg_of_blk.pop(bb)
                            rec = sb_w.tile([P, H, 1], f32, tag="rec")
                            nc.vector.reciprocal(
                                rec[:].rearrange("p h o -> p (h o)"),
                                agg[:, HC:MSGW])
                            xb = sb_w.tile([P, HC], f32, tag="xb")
                            nc.vector.tensor_tensor(
                                out=xb[:].rearrange("p (h c) -> p h c", h=H),
                                in0=agg[:, 0:HC].rearrange("p (h c) -> p h c", h=H),
                                in1=rec[:].to_broadcast([P, H, Ch]),
                                op=mybir.AluOpType.mult)
                            nc.vector.tensor_scalar_max(xb[:], xb[:], 0.0)
                            if l < 2:
                                xps = ps_xp.tile([P, P], f32, tag="xps")
                                nc.tensor.transpose(xps[:], xb[:], ident[:])
                                nc.vector.tensor_copy(
                                    out=xT[:, bb * P:(bb + 1) * P], in_=xps[:])
                            else:
                                bsel = sb_w.tile([P, GRAPHS], f32, tag="bsel")
                                nc.vector.tensor_tensor(
                                    out=bsel[:],
                                    in0=batchf[:, bb, :].to_broadcast([P, GRAPHS]),
                                    in1=iota[:, 0, :GRAPHS],
                                    op=mybir.AluOpType.is_equal)
                                nc.tensor.matmul(pooled_ps[:], lhsT=bsel[:],
                                                 rhs=xb[:], start=(bb == 0),
                                                 stop=(bb == nb - 1))

        # ===== head =====
        pooled_sb = sb.tile([GRAPHS, HC], f32)
        nc.vector.tensor_copy(out=pooled_sb[:], in_=pooled_ps[:])
        pT_ps = ps_xp.tile([P, GRAPHS], f32, tag="xps")
        nc.tensor.transpose(pT_ps[:], pooled_sb[:], ident[:GRAPHS, :GRAPHS])
        pT_sb = sb.tile([P, GRAPHS], f32)
        nc.vector.tensor_copy(out=pT_sb[:], in_=pT_ps[:])
        log_ps = ps_xp.tile([GRAPHS, OUT], f32, tag="logps")
        nc.tensor.matmul(log_ps[:], lhsT=pT_sb[:], rhs=wh[:], start=True, stop=True)
        log_sb = sb.tile([GRAPHS, OUT], f32)
        nc.vector.tensor_copy(out=log_sb[:], in_=log_ps[:])
        nc.sync.dma_start(out=out_d[:], in_=log_sb[:])

    _fixup_wait_limits(nc)
    mybir.codegen_inst_isa_subclasses(nc)
    return nc


def prepare(x, Ws, a_srcs, a_dsts, biases, Wh, bh, edge_index, batch):
    n = x.shape[0]
    npad = int(math.ceil(n / (NCORES * P)) * NCORES * P)
    per = npad // NCORES
    nb = per // P
    bank = (npad // 2 + P - 1) // P * P   # bank boundary (block aligned)

    x = np.asarray(x, np.float32)
    Ws = [np.asarray(w, np.float32) for w in Ws]
    a_srcs = [np.asarray(a, np.float32) for a in a_srcs]
    a_dsts = [np.asarray(a, np.float32) for a in a_dsts]
    Wh = np.asarray(Wh, np.float32)
    bh = np.asarray(bh, np.float32)
    edge_index = np.asarray(edge_index)
    batch = np.asarray(batch)
    for b in biases:
        assert np.allclose(np.asarray(b), 0.0), "nonzero GAT biases unsupported"

    # W_aug = [W@Ad | W@As | W]  (row layout [a_d | a_s | h])
    waugs = []
    for l in range(3):
        As = np.zeros((HC, H), np.float32)
        Ad = np.zeros((HC, H), np.float32)
        for h in range(H):
            As[h * Ch:(h + 1) * Ch, h] = a_srcs[l][h]
            Ad[h * Ch:(h + 1) * Ch, h] = a_dsts[l][h]
        W = Ws[l]
        waugs.append(np.concatenate([W @ Ad, W @ As, W], axis=1))
    waug = np.stack(waugs, 0)  # [3, 128, AUG]

    # edges + self loops (incl. pad nodes, so every row has >=1 edge)
    src_all = np.concatenate([edge_index[0].astype(np.int64),
                              np.arange(npad, dtype=np.int64)])
    dst_all = np.concatenate([edge_index[1].astype(np.int64),
                              np.arange(npad, dtype=np.int64)])
    plan = _prep_edges(src_all, dst_all, per, nb, bank)

    xpad = np.zeros((npad, HC), np.float32)
    xpad[:n] = x
    iota = np.tile(np.arange(P, dtype=np.float32)[None, :], (P, 4)).astype(BF16)

    batchf_full = np.full(npad, -1.0, np.float32)
    batchf_full[:n] = batch.astype(np.float32)

    nc = _build(npad, bank, plan)

    in_maps = []
    for c in range(NCORES):
        sl = slice(c * per, (c + 1) * per)
        in_maps.append({
            "xT": np.ascontiguousarray(xpad[sl].T),
            "waug": waug,
            "wh": Wh,
            "iota": iota,
            "srci": plan.srcw[c],
            "adi": plan.adw[c],
            "dloc": plan.dlocs[c],
            "batchf": np.ascontiguousarray(
                batchf_full[sl].reshape(nb, P).T).astype(BF16),
            })
    return nc, in_maps


def run_gat(x, Ws, a_srcs, a_dsts, biases, Wh, bh, edge_index, batch):
    nc, in_maps = prepare(x, Ws, a_srcs, a_dsts, biases, Wh, bh,
                          edge_index, batch)
    res = run_bass_kernel_spmd(nc, in_maps, list(range(NCORES)))
    global LAST_EXEC_NS
    LAST_EXEC_NS = getattr(res, "exec_time_ns", None)
    logits = np.zeros((GRAPHS, OUT), np.float32)
    for c in range(NCORES):
        logits += res.results[c]["out"]
    return logits + bh


def kernel(**inputs):
    return np.asarray(run_gat(
        inputs["x"], inputs["Ws"], inputs["a_srcs"], inputs["a_dsts"],
        inputs["biases"], inputs["Wh"], inputs["bh"], inputs["edge_index"],
        inputs["batch"]), np.float32)


# revision 9
# speedup vs baseline: 1.2454x; 1.2454x over previous
"""Trainium2 Bass kernel for 3-layer GAT + global_add_pool + linear head.

Sharding: nodes (and incoming edges) partitioned across 8 cores by dst; the
per-layer node table (rows [a_d | a_s | h], bf16, 136-elem rows) is exchanged
with an AllGather; per-edge source rows are fetched with per-tile indirect
DMAs (128 x 272B descriptors per instruction, int32 global indices, edges
sorted by src within each dst block for DRAM locality). Per-edge a_dst is
produced on the idle PE: transpose the bf16 edge->slot selector and multiply
against the local table's a_d columns. The segment softmax + weighted
aggregation run as bf16 selector matmuls with fp32 PSUM accumulation per
128-node dst block. LeakyReLU runs on the vector engine ((x*0.2) max x), Exp
is the only scalar-engine table function; PSUM->SBUF selector copies ride the
otherwise idle scalar engine. Partial pooled logits are summed on the host.

kernel() is self-contained: shapes hardcoded, no file reads.
"""
import math
import numpy as np
from contextlib import ExitStack

import ml_dtypes

import concourse.bass as bass
import concourse.mybir as mybir
import concourse.tile as tile
from concourse.bass import IndirectOffsetOnAxis
from concourse.bass_utils import run_bass_kernel_spmd
from concourse.tile_rust import add_dep_helper
from concourse.masks import make_identity

NCORES = 8
P = 128
H = 4
Ch = 32
HC = 128            # H * Ch
AUG = HC + 2 * H    # 136: [a_d | a_s | h] table row
MSGW = HC + H       # 132: [msg | ex]
NEG_SLOPE = 0.2
GRAPHS = 64
OUT = 10
CHUNK_TILES = 32    # tiles (of 128 edges) per gather buffer rotation

BF16 = ml_dtypes.bfloat16

# instruction types whose BIR struct cannot carry all Tile-emitted waits
_WAIT_CAPS = {
    "InstNoOp": 1,
    "InstDrain": 1,
    "InstCollectiveCompute": 1,
}


def _fixup_wait_limits(nc):
    k = 0
    for fn in nc.m.functions:
        for blk in fn.blocks:
            out = []
            for inst in blk.instructions:
                cap = _WAIT_CAPS.get(type(inst).__name__, 1)
                si = inst.sync_info
                if si is not None:
                    waits = list(si.on_wait)
                    if len(waits) > cap:
                        keep, move = waits[:cap], waits[cap:]
                        for w in move:
                            nop = mybir.InstNoOp(name=f"waitfix_{k}", text_hint="wait_fixup")
                            k += 1
                            nop.engine = inst.engine
                            nop.sync_info = type(si)(on_wait=[w], on_update=[])
                            out.append(nop)
                        inst.sync_info = type(si)(on_wait=list(keep), on_update=list(si.on_update))
                out.append(inst)
            blk.instructions = out
    return k


class EdgePlan:
    __slots__ = ("tiles", "Tpad", "srcs", "dlocs")


def _prep_edges(src_all, dst_all, per, nb):
    """Per-core edge tiling (uniform across cores). Edges grouped by dst
    block, sorted by src within the block. Tiles padded per block to the max
    core's count; tile sequence padded to a CHUNK_TILES multiple."""
    core = dst_all // per
    loc = dst_all % per
    blk = loc // P

    per_core = []
    tiles_b = np.zeros(nb, np.int64)
    for c in range(NCORES):
        m = core == c
        s, d, b = src_all[m], loc[m], blk[m]
        blocks = []
        for bb in range(nb):
            mb = b == bb
            sb_, db_ = s[mb], d[mb]
            order = np.argsort(sb_, kind="stable")
            blocks.append((sb_[order], db_[order]))
            tiles_b[bb] = max(tiles_b[bb], (len(sb_) + P - 1) // P)
        per_core.append(blocks)

    tiles = []          # (block, k) ; block=-1 pad
    for bb in range(nb):
        for k in range(int(tiles_b[bb])):
            tiles.append((bb, k))
    while len(tiles) % CHUNK_TILES:
        tiles.append((-1, 0))
    Tpad = len(tiles)

    first_t, last_t = {}, {}
    for t, (bb, k) in enumerate(tiles):
        if bb < 0:
            continue
        first_t.setdefault(bb, t)
        last_t[bb] = t
    tile_meta = [(bb, bb >= 0 and first_t[bb] == t, bb >= 0 and last_t[bb] == t)
                 for t, (bb, k) in enumerate(tiles)]

    plan = EdgePlan()
    plan.tiles = tile_meta
    plan.Tpad = Tpad
    plan.srcs = []
    plan.dlocs = []
    for c in range(NCORES):
        src_idx = np.zeros((P, Tpad), np.int32)
        dloc_f = np.full((P, Tpad), -1.0, np.float32)
        for t, (bb, k) in enumerate(tiles):
            if bb < 0:
                continue
            s, d = per_core[c][bb]
            seg_s = s[k * P:(k + 1) * P]
            seg_d = d[k * P:(k + 1) * P]
            n = len(seg_s)
            if n:
                src_idx[:n, t] = seg_s
                dloc_f[:n, t] = (seg_d % P).astype(np.float32)
        plan.srcs.append(src_idx)
        plan.dlocs.append(dloc_f.astype(BF16))
    return plan


def _build(npad, plan):
    per = npad // NCORES
    nb = per // P
    nlayers = 3
    f32 = mybir.dt.float32
    bf16 = mybir.dt.bfloat16
    Tpad = plan.Tpad
    nchunks = Tpad // CHUNK_TILES

    nc = bass.Bass(num_devices=NCORES)
    # ---- dram I/O
    xT_d = nc.dram_tensor("xT", [P, per], f32, kind="ExternalInput")
    waug_d = nc.dram_tensor("waug", [nlayers, P, AUG], f32, kind="ExternalInput")
    wh_d = nc.dram_tensor("wh", [P, OUT], f32, kind="ExternalInput")
    iota_d = nc.dram_tensor("iota", [P, 4 * P], bf16, kind="ExternalInput")
    identb_d = nc.dram_tensor("identb", [P, P], bf16, kind="ExternalInput")
    srci_d = nc.dram_tensor("srci", [P, Tpad], mybir.dt.int32, kind="ExternalInput")
    dloc_d = nc.dram_tensor("dloc", [P, Tpad], bf16, kind="ExternalInput")
    batchf_d = nc.dram_tensor("batchf", [P, nb], bf16, kind="ExternalInput")
    out_d = nc.dram_tensor("out", [GRAPHS, OUT], f32, kind="ExternalOutput")

    h_loc = [nc.dram_tensor(f"h_loc{l}", [per, AUG], bf16) for l in range(nlayers)]
    h_full = [nc.dram_tensor(f"h_full{l}", [npad, AUG], bf16, addr_space="Shared")
              for l in range(nlayers)]

    groups = [list(range(NCORES))]

    with ExitStack() as ctx:
        tc = ctx.enter_context(tile.TileContext(nc))
        sb = ctx.enter_context(tc.tile_pool(name="sb", bufs=1))
        sb_g = ctx.enter_context(tc.tile_pool(name="sbg", bufs=2))
        sb_w = ctx.enter_context(tc.tile_pool(name="sbw", bufs=3))
        ps_h = ctx.enter_context(tc.tile_pool(name="psh", bufs=1, space="PSUM"))
        ps_agg = ctx.enter_context(tc.tile_pool(name="psagg", bufs=2, space="PSUM"))
        ps_st = ctx.enter_context(tc.tile_pool(name="psst", bufs=1, space="PSUM"))
        ps_xp = ctx.enter_context(tc.tile_pool(name="psxp", bufs=1, space="PSUM"))
        ps_fin = ctx.enter_context(tc.tile_pool(name="psfin", bufs=1, space="PSUM"))

        # ---- persistent SBUF state
        xT = sb.tile([P, per], f32)
        nc.sync.dma_start(out=xT[:], in_=xT_d[:])
        waug = sb.tile([P, nlayers, AUG], f32)
        nc.sync.dma_start(out=waug[:], in_=waug_d[:].rearrange("l p a -> p l a"))
        wh = sb.tile([P, OUT], f32)
        nc.sync.dma_start(out=wh[:], in_=wh_d[:])
        iota = sb.tile([P, 4, P], bf16)
        nc.sync.dma_start(out=iota[:].rearrange("p a b -> p (a b)"), in_=iota_d[:])
        identb = sb.tile([P, P], bf16)
        nc.sync.dma_start(out=identb[:], in_=identb_d[:])
        srci = sb.tile([P, Tpad], mybir.dt.int32)
        nc.sync.dma_start(out=srci[:], in_=srci_d[:])
        dloc = sb.tile([P, Tpad, 1], bf16)
        nc.sync.dma_start(out=dloc[:].rearrange("p t o -> p (t o)"), in_=dloc_d[:])
        batchf = sb.tile([P, nb, 1], bf16)
        nc.sync.dma_start(out=batchf[:].rearrange("p b o -> p (b o)"), in_=batchf_d[:])
        ident = sb.tile([P, P], f32)
        make_identity(nc, ident[:])

        hsb = sb.tile([P, nb, AUG], bf16)
        pooled_ps = ps_fin.tile([GRAPHS, HC], f32)

        for l in range(3):
            # ===== node phase: rows [a_d | a_s | h] = x @ [W@Ad | W@As | W] =====
            for b in range(nb):
                ps = ps_h.tile([P, AUG], f32)
                nc.tensor.matmul(ps[:], lhsT=xT[:, b * P:(b + 1) * P],
                                 rhs=waug[:, l, :], start=True, stop=True)
                nc.vector.tensor_copy(out=hsb[:, b, :], in_=ps[:])
            dh = nc.sync.dma_start(
                out=h_loc[l][:].rearrange("(b p) d -> p b d", p=P),
                in_=hsb[:])
            cch = nc.gpsimd.collective_compute(
                "AllGather", mybir.AluOpType.bypass, replica_groups=groups,
                ins=[h_loc[l][:]], outs=[h_full[l][:]])
            add_dep_helper(cch.ins, dh.ins, sync=True, reason="h write before ag")

            # ===== edge phase =====
            agg_of_blk = {}
            for cidx in range(nchunks):
                t0 = cidx * CHUNK_TILES
                g = sb_g.tile([P, CHUNK_TILES, AUG], bf16, tag="gath")
                for j in range(CHUNK_TILES):
                    gi = nc.gpsimd.indirect_dma_start(
                        out=g[:, j, :], out_offset=None, in_=h_full[l][:],
                        in_offset=IndirectOffsetOnAxis(
                            ap=srci[:, t0 + j:t0 + j + 1], axis=0))
                    add_dep_helper(gi.ins, cch.ins, sync=True, reason="gather after ag")

                for q in range(CHUNK_TILES // 4):
                    tq = t0 + 4 * q
                    if all(plan.tiles[tq + j][0] < 0 for j in range(4)):
                        continue
                    sel4 = sb_w.tile([P, 4, P], bf16, tag="sel")
                    nc.vector.tensor_tensor(
                        out=sel4[:],
                        in0=dloc[:, tq:tq + 4, :].to_broadcast([P, 4, P]),
                        in1=iota[:], op=mybir.AluOpType.is_equal)
                    lg4 = sb_w.tile([P, 4, H], f32, tag="lg")
                    for j in range(4):
                        t = tq + j
                        bb, st, sp = plan.tiles[t]
                        if bb < 0:
                            continue
                        stp = ps_st.tile([P, P], bf16, tag="selT", name=f"st{l}_{t}")
                        nc.tensor.transpose(stp[:], sel4[:, j, :], identb[:])
                        sts = sb_w.tile([P, P], bf16, tag="selTs", name=f"sts{l}_{t}")
                        nc.scalar.copy(out=sts[:], in_=stp[:])
                        adp = ps_st.tile([P, H], f32, tag="ad4", name=f"ad{l}_{t}")
                        nc.tensor.matmul(adp[:], lhsT=sts[:], rhs=hsb[:, bb, 0:H],
                                         start=True, stop=True)
                        nc.vector.tensor_tensor(
                            out=lg4[:, j, :], in0=g[:, 4 * q + j, H:2 * H],
                            in1=adp[:], op=mybir.AluOpType.add)
                    lr4 = sb_w.tile([P, 4, H], f32, tag="lr")
                    nc.vector.scalar_tensor_tensor(
                        out=lr4[:], in0=lg4[:], scalar=NEG_SLOPE, in1=lg4[:],
                        op0=mybir.AluOpType.mult, op1=mybir.AluOpType.max)
                    ex4 = sb_w.tile([P, 4, H, 1], bf16, tag="ex")
                    nc.scalar.activation(ex4[:].rearrange("p a b o -> p a (b o)"),
                                         lr4[:],
                                         mybir.ActivationFunctionType.Exp)
                    msg4 = sb_w.tile([P, 4, MSGW], bf16, tag="msg")
                    nc.vector.tensor_tensor(
                        out=msg4[:, :, 0:HC].rearrange("p q (h c) -> p q h c", h=H),
                        in0=g[:, 4 * q:4 * q + 4, 2 * H:2 * H + HC]
                            .rearrange("p q (h c) -> p q h c", h=H),
                        in1=ex4[:].to_broadcast([P, 4, H, Ch]),
                        op=mybir.AluOpType.mult)
                    nc.vector.tensor_copy(
                        out=msg4[:, :, HC:MSGW],
                        in_=ex4[:].rearrange("p a b o -> p a (b o)"))
                    for j in range(4):
                        t = tq + j
                        bb, st, sp = plan.tiles[t]
                        if bb < 0:
                            continue
                        if st:
                            agg_of_blk[bb] = ps_agg.tile(
                                [P, MSGW], f32, tag="agg", name=f"agg{l}_{bb}")
                        nc.tensor.matmul(agg_of_blk[bb][:], lhsT=sel4[:, j, :],
                                         rhs=msg4[:, j, :],
                                         start=bool(st), stop=bool(sp))
                        if sp:
                            agg = agg_of_blk.pop(bb)
                            rec = sb_w.tile([P, H, 1], f32, tag="rec")
                            nc.vector.reciprocal(
                                rec[:].rearrange("p h o -> p (h o)"),
                                agg[:, HC:MSGW])
                            xb = sb_w.tile([P, HC], f32, tag="xb")
                            nc.vector.tensor_tensor(
                                out=xb[:].rearrange("p (h c) -> p h c", h=H),
                                in0=agg[:, 0:HC].rearrange("p (h c) -> p h c", h=H),
                                in1=rec[:].to_broadcast([P, H, Ch]),
                                op=mybir.AluOpType.mult)
                            nc.vector.tensor_scalar_max(xb[:], xb[:], 0.0)
                            if l < 2:
                                xps = ps_xp.tile([P, P], f32, tag="xps")
                                nc.tensor.transpose(xps[:], xb[:], ident[:])
                                nc.vector.tensor_copy(
                                    out=xT[:, bb * P:(bb + 1) * P], in_=xps[:])
                            else:
                                bsel = sb_w.tile([P, GRAPHS], f32, tag="bsel")
                                nc.vector.tensor_tensor(
                                    out=bsel[:],
                                    in0=batchf[:, bb, :].to_broadcast([P, GRAPHS]),
                                    in1=iota[:, 0, :GRAPHS],
                                    op=mybir.AluOpType.is_equal)
                                nc.tensor.matmul(pooled_ps[:], lhsT=bsel[:],
                                                 rhs=xb[:], start=(bb == 0),
                                                 stop=(bb == nb - 1))

        # ===== head =====
        pooled_sb = sb.tile([GRAPHS, HC], f32)
        nc.vector.tensor_copy(out=pooled_sb[:], in_=pooled_ps[:])
        pT_ps = ps_xp.tile([P, GRAPHS], f32, tag="xps")
        nc.tensor.transpose(pT_ps[:], pooled_sb[:], ident[:GRAPHS, :GRAPHS])
        pT_sb = sb.tile([P, GRAPHS], f32)
        nc.vector.tensor_copy(out=pT_sb[:], in_=pT_ps[:])
        log_ps = ps_xp.tile([GRAPHS, OUT], f32, tag="logps")
        nc.tensor.matmul(log_ps[:], lhsT=pT_sb[:], rhs=wh[:], start=True, stop=True)
        log_sb = sb.tile([GRAPHS, OUT], f32)
        nc.vector.tensor_copy(out=log_sb[:], in_=log_ps[:])
        nc.sync.dma_start(out=out_d[:], in_=log_sb[:])

    _fixup_wait_limits(nc)
    return nc


def prepare(x, Ws, a_srcs, a_dsts, biases, Wh, bh, edge_index, batch):
    n = x.shape[0]
    npad = int(math.ceil(n / (NCORES * P)) * NCORES * P)
    per = npad // NCORES
    nb = per // P

    x = np.asarray(x, np.float32)
    Ws = [np.asarray(w, np.float32) for w in Ws]
    a_srcs = [np.asarray(a, np.float32) for a in a_srcs]
    a_dsts = [np.asarray(a, np.float32) for a in a_dsts]
    Wh = np.asarray(Wh, np.float32)
    bh = np.asarray(bh, np.float32)
    edge_index = np.asarray(edge_index)
    batch = np.asarray(batch)
    for b in biases:
        assert np.allclose(np.asarray(b), 0.0), "nonzero GAT biases unsupported"

    # W_aug = [W@Ad | W@As | W]  (row layout [a_d | a_s | h])
    waugs = []
    for l in range(3):
        As = np.zeros((HC, H), np.float32)
        Ad = np.zeros((HC, H), np.float32)
        for h in range(H):
            As[h * Ch:(h + 1) * Ch, h] = a_srcs[l][h]
            Ad[h * Ch:(h + 1) * Ch, h] = a_dsts[l][h]
        W = Ws[l]
        waugs.append(np.concatenate([W @ Ad, W @ As, W], axis=1))
    waug = np.stack(waugs, 0)  # [3, 128, AUG]

    # edges + self loops (incl. pad nodes, so every row has >=1 edge)
    src_all = np.concatenate([edge_index[0].astype(np.int64),
                              np.arange(npad, dtype=np.int64)])
    dst_all = np.concatenate([edge_index[1].astype(np.int64),
                              np.arange(npad, dtype=np.int64)])
    plan = _prep_edges(src_all, dst_all, per, nb)

    xpad = np.zeros((npad, HC), np.float32)
    xpad[:n] = x
    iota = np.tile(np.arange(P, dtype=np.float32)[None, :], (P, 4)).astype(BF16)
    identb = np.eye(P, dtype=np.float32).astype(BF16)

    batchf_full = np.full(npad, -1.0, np.float32)
    batchf_full[:n] = batch.astype(np.float32)

    nc = _build(npad, plan)

    in_maps = []
    for c in range(NCORES):
        sl = slice(c * per, (c + 1) * per)
        in_maps.append({
            "xT": np.ascontiguousarray(xpad[sl].T),
            "waug": waug,
            "wh": Wh,
            "iota": iota,
            "identb": identb,
            "srci": plan.srcs[c],
            "dloc": plan.dlocs[c],
            "batchf": np.ascontiguousarray(
                batchf_full[sl].reshape(nb, P).T).astype(BF16),
            })
    return nc, in_maps


def run_gat(x, Ws, a_srcs, a_dsts, biases, Wh, bh, edge_index, batch):
    nc, in_maps = prepare(x, Ws, a_srcs, a_dsts, biases, Wh, bh,
                          edge_index, batch)
    res = run_bass_kernel_spmd(nc, in_maps, list(range(NCORES)))
    global LAST_EXEC_NS
    LAST_EXEC_NS = getattr(res, "exec_time_ns", None)
    logits = np.zeros((GRAPHS, OUT), np.float32)
    for c in range(NCORES):
        logits += res.results[c]["out"]
    return logits + bh


def kernel(**inputs):
    return np.asarray(run_gat(
        inputs["x"], inputs["Ws"], inputs["a_srcs"], inputs["a_dsts"],
        inputs["biases"], inputs["Wh"], inputs["bh"], inputs["edge_index"],
        inputs["batch"]), np.float32)


# revision 13
# speedup vs baseline: 1.5173x; 1.2183x over previous
"""Trainium2 Bass kernel for 3-layer GAT + global_add_pool + linear head.

Sharding: nodes (and incoming edges) partitioned across 8 cores by dst; the
per-layer node table (rows [a_d | a_s | h], bf16, 136-elem rows) is exchanged
with an AllGather; per-edge source rows are fetched with per-tile indirect
DMAs (128 x 272B descriptors per instruction, int32 global indices, edges
sorted by src within each dst block for DRAM locality). Per-edge a_dst is
produced on the idle PE: transpose the bf16 edge->slot selector and multiply
against the local table's a_d columns. The segment softmax + weighted
aggregation run as bf16 selector matmuls with fp32 PSUM accumulation per
128-node dst block. LeakyReLU runs on the vector engine ((x*0.2) max x), Exp
is the only scalar-engine table function; PSUM->SBUF selector copies ride the
otherwise idle scalar engine. Partial pooled logits are summed on the host.

kernel() is self-contained: shapes hardcoded, no file reads.
"""
import math
import numpy as np
from contextlib import ExitStack

import ml_dtypes

import concourse.bass as bass
import concourse.mybir as mybir
import concourse.tile as tile
from concourse.bass import IndirectOffsetOnAxis
from concourse.bass_utils import run_bass_kernel_spmd
from concourse.tile_rust import add_dep_helper
from concourse.masks import make_identity

NCORES = 8
P = 128
H = 4
Ch = 32
HC = 128            # H * Ch
AUG = HC + 2 * H    # 136: [a_d | a_s | h] table row
MSGW = HC + H       # 132: [msg | ex]
NEG_SLOPE = 0.2
GRAPHS = 64
OUT = 10
CHUNK_TILES = 32    # tiles (of 128 edges) per gather buffer rotation

BF16 = ml_dtypes.bfloat16

# instruction types whose BIR struct cannot carry all Tile-emitted waits
_WAIT_CAPS = {
    "InstNoOp": 1,
    "InstDrain": 1,
    "InstCollectiveCompute": 1,
}


def _fixup_wait_limits(nc):
    k = 0
    for fn in nc.m.functions:
        for blk in fn.blocks:
            out = []
            for inst in blk.instructions:
                cap = _WAIT_CAPS.get(type(inst).__name__, 1)
                si = inst.sync_info
                if si is not None:
                    waits = list(si.on_wait)
                    if len(waits) > cap:
                        keep, move = waits[:cap], waits[cap:]
                        for w in move:
                            nop = mybir.InstNoOp(name=f"waitfix_{k}", text_hint="wait_fixup")
                            k += 1
                            nop.engine = inst.engine
                            nop.sync_info = type(si)(on_wait=[w], on_update=[])
                            out.append(nop)
                        inst.sync_info = type(si)(on_wait=list(keep), on_update=list(si.on_update))
                out.append(inst)
            blk.instructions = out
    return k


class EdgePlan:
    __slots__ = ("tiles", "Tpad", "srcs", "dlocs")


def _prep_edges(src_all, dst_all, per, nb):
    """Per-core edge tiling (uniform across cores). Edges grouped by dst
    block, sorted by src within the block. Tiles padded per block to the max
    core's count; tile sequence padded to a CHUNK_TILES multiple."""
    core = dst_all // per
    loc = dst_all % per
    blk = loc // P

    per_core = []
    tiles_b = np.zeros(nb, np.int64)
    for c in range(NCORES):
        m = core == c
        s, d, b = src_all[m], loc[m], blk[m]
        blocks = []
        for bb in range(nb):
            mb = b == bb
            sb_, db_ = s[mb], d[mb]
            order = np.argsort(sb_, kind="stable")
            blocks.append((sb_[order], db_[order]))
            tiles_b[bb] = max(tiles_b[bb], (len(sb_) + P - 1) // P)
        per_core.append(blocks)

    tiles = []          # (block, k) ; block=-1 pad
    for bb in range(nb):
        for k in range(int(tiles_b[bb])):
            tiles.append((bb, k))
    while len(tiles) % CHUNK_TILES:
        tiles.append((-1, 0))
    Tpad = len(tiles)

    first_t, last_t = {}, {}
    for t, (bb, k) in enumerate(tiles):
        if bb < 0:
            continue
        first_t.setdefault(bb, t)
        last_t[bb] = t
    tile_meta = [(bb, bb >= 0 and first_t[bb] == t, bb >= 0 and last_t[bb] == t)
                 for t, (bb, k) in enumerate(tiles)]

    plan = EdgePlan()
    plan.tiles = tile_meta
    plan.Tpad = Tpad
    plan.srcs = []
    plan.dlocs = []
    for c in range(NCORES):
        src_idx = np.zeros((P, Tpad), np.int32)
        dloc_f = np.full((P, Tpad), -1.0, np.float32)
        for t, (bb, k) in enumerate(tiles):
            if bb < 0:
                continue
            s, d = per_core[c][bb]
            seg_s = s[k * P:(k + 1) * P]
            seg_d = d[k * P:(k + 1) * P]
            n = len(seg_s)
            if n:
                src_idx[:n, t] = seg_s
                dloc_f[:n, t] = (seg_d % P).astype(np.float32)
        plan.srcs.append(src_idx)
        plan.dlocs.append(dloc_f.astype(BF16))
    return plan


def _build(npad, plan):
    per = npad // NCORES
    nb = per // P
    nlayers = 3
    f32 = mybir.dt.float32
    bf16 = mybir.dt.bfloat16
    Tpad = plan.Tpad
    nchunks = Tpad // CHUNK_TILES

    nc = bass.Bass(num_devices=NCORES)
    # ---- dram I/O
    xT_d = nc.dram_tensor("xT", [P, per], f32, kind="ExternalInput")
    waug_d = nc.dram_tensor("waug", [nlayers, P, AUG], f32, kind="ExternalInput")
    wh_d = nc.dram_tensor("wh", [P, OUT], f32, kind="ExternalInput")
    iota_d = nc.dram_tensor("iota", [P, 4 * P], bf16, kind="ExternalInput")
    identb_d = nc.dram_tensor("identb", [P, P], bf16, kind="ExternalInput")
    srci_d = nc.dram_tensor("srci", [P, Tpad], mybir.dt.int32, kind="ExternalInput")
    dloc_d = nc.dram_tensor("dloc", [P, Tpad], bf16, kind="ExternalInput")
    batchf_d = nc.dram_tensor("batchf", [P, nb], bf16, kind="ExternalInput")
    out_d = nc.dram_tensor("out", [GRAPHS, OUT], f32, kind="ExternalOutput")

    h_loc = [nc.dram_tensor(f"h_loc{l}", [per, AUG], bf16) for l in range(nlayers)]
    h_full = [nc.dram_tensor(f"h_full{l}", [npad, AUG], bf16, addr_space="Shared")
              for l in range(nlayers)]

    groups = [list(range(NCORES))]

    with ExitStack() as ctx:
        tc = ctx.enter_context(tile.TileContext(nc))
        sb = ctx.enter_context(tc.tile_pool(name="sb", bufs=1))
        sb_g = ctx.enter_context(tc.tile_pool(name="sbg", bufs=3))
        sb_w = ctx.enter_context(tc.tile_pool(name="sbw", bufs=3))
        ps_h = ctx.enter_context(tc.tile_pool(name="psh", bufs=1, space="PSUM"))
        ps_agg = ctx.enter_context(tc.tile_pool(name="psagg", bufs=2, space="PSUM"))
        ps_st = ctx.enter_context(tc.tile_pool(name="psst", bufs=1, space="PSUM"))
        ps_xp = ctx.enter_context(tc.tile_pool(name="psxp", bufs=1, space="PSUM"))
        ps_fin = ctx.enter_context(tc.tile_pool(name="psfin", bufs=1, space="PSUM"))

        # ---- persistent SBUF state
        xT = sb.tile([P, per], f32)
        nc.sync.dma_start(out=xT[:], in_=xT_d[:])
        waug = sb.tile([P, nlayers, AUG], f32)
        nc.sync.dma_start(out=waug[:], in_=waug_d[:].rearrange("l p a -> p l a"))
        wh = sb.tile([P, OUT], f32)
        nc.sync.dma_start(out=wh[:], in_=wh_d[:])
        iota = sb.tile([P, 4, P], bf16)
        nc.sync.dma_start(out=iota[:].rearrange("p a b -> p (a b)"), in_=iota_d[:])
        identb = sb.tile([P, P], bf16)
        nc.sync.dma_start(out=identb[:], in_=identb_d[:])
        srci = sb.tile([P, Tpad], mybir.dt.int32)
        nc.sync.dma_start(out=srci[:], in_=srci_d[:])
        dloc = sb.tile([P, Tpad, 1], bf16)
        nc.sync.dma_start(out=dloc[:].rearrange("p t o -> p (t o)"), in_=dloc_d[:])
        batchf = sb.tile([P, nb, 1], bf16)
        nc.sync.dma_start(out=batchf[:].rearrange("p b o -> p (b o)"), in_=batchf_d[:])
        ident = sb.tile([P, P], f32)
        make_identity(nc, ident[:])

        hsb = sb.tile([P, nb, AUG], bf16)
        pooled_ps = ps_fin.tile([GRAPHS, HC], f32)

        for l in range(3):
            # ===== node phase: rows [a_d | a_s | h] = x @ [W@Ad | W@As | W] =====
            for b in range(nb):
                ps = ps_h.tile([P, AUG], f32)
                nc.tensor.matmul(ps[:], lhsT=xT[:, b * P:(b + 1) * P],
                                 rhs=waug[:, l, :], start=True, stop=True)
                nc.vector.tensor_copy(out=hsb[:, b, :], in_=ps[:])
            dh = nc.sync.dma_start(
                out=h_loc[l][:].rearrange("(b p) d -> p b d", p=P),
                in_=hsb[:])
            cch = nc.gpsimd.collective_compute(
                "AllGather", mybir.AluOpType.bypass, replica_groups=groups,
                ins=[h_loc[l][:]], outs=[h_full[l][:]])
            add_dep_helper(cch.ins, dh.ins, sync=True, reason="h write before ag")

            # ===== edge phase =====
            agg_of_blk = {}
            for cidx in range(nchunks):
                t0 = cidx * CHUNK_TILES
                g = sb_g.tile([P, CHUNK_TILES, AUG], bf16, tag="gath")
                for j in range(CHUNK_TILES):
                    gi = nc.gpsimd.indirect_dma_start(
                        out=g[:, j, :], out_offset=None, in_=h_full[l][:],
                        in_offset=IndirectOffsetOnAxis(
                            ap=srci[:, t0 + j:t0 + j + 1], axis=0))
                    if j == 0:
                        # Pool program order covers the rest of the chunk
                        add_dep_helper(gi.ins, cch.ins, sync=True,
                                       reason="gather after ag")

                for q in range(CHUNK_TILES // 4):
                    tq = t0 + 4 * q
                    if all(plan.tiles[tq + j][0] < 0 for j in range(4)):
                        continue
                    sel4 = sb_w.tile([P, 4, P], bf16, tag="sel")
                    nc.vector.tensor_tensor(
                        out=sel4[:],
                        in0=dloc[:, tq:tq + 4, :].to_broadcast([P, 4, P]),
                        in1=iota[:], op=mybir.AluOpType.is_equal)
                    lg4 = sb_w.tile([P, 4, H], f32, tag="lg")
                    for j in range(4):
                        t = tq + j
                        bb, st, sp = plan.tiles[t]
                        if bb < 0:
                            continue
                        stp = ps_st.tile([P, P], bf16, tag="selT", name=f"st{l}_{t}")
                        nc.tensor.transpose(stp[:], sel4[:, j, :], identb[:])
                        sts = sb_w.tile([P, P], bf16, tag="selTs", name=f"sts{l}_{t}")
                        nc.scalar.copy(out=sts[:], in_=stp[:])
                        adp = ps_st.tile([P, H], f32, tag="ad4", name=f"ad{l}_{t}")
                        nc.tensor.matmul(adp[:], lhsT=sts[:], rhs=hsb[:, bb, 0:H],
                                         start=True, stop=True)
                        nc.vector.tensor_tensor(
                            out=lg4[:, j, :], in0=g[:, 4 * q + j, H:2 * H],
                            in1=adp[:], op=mybir.AluOpType.add)
                    lr4 = sb_w.tile([P, 4, H], f32, tag="lr")
                    nc.vector.scalar_tensor_tensor(
                        out=lr4[:], in0=lg4[:], scalar=NEG_SLOPE, in1=lg4[:],
                        op0=mybir.AluOpType.mult, op1=mybir.AluOpType.max)
                    ex4 = sb_w.tile([P, 4, H, 1], bf16, tag="ex")
                    nc.scalar.activation(ex4[:].rearrange("p a b o -> p a (b o)"),
                                         lr4[:],
                                         mybir.ActivationFunctionType.Exp)
                    msg4 = sb_w.tile([P, 4, MSGW], bf16, tag="msg")
                    nc.vector.tensor_tensor(
                        out=msg4[:, :, 0:HC].rearrange("p q (h c) -> p q h c", h=H),
                        in0=g[:, 4 * q:4 * q + 4, 2 * H:2 * H + HC]
                            .rearrange("p q (h c) -> p q h c", h=H),
                        in1=ex4[:].to_broadcast([P, 4, H, Ch]),
                        op=mybir.AluOpType.mult)
                    nc.vector.tensor_copy(
                        out=msg4[:, :, HC:MSGW],
                        in_=ex4[:].rearrange("p a b o -> p a (b o)"))
                    for j in range(4):
                        t = tq + j
                        bb, st, sp = plan.tiles[t]
                        if bb < 0:
                            continue
                        if st:
                            agg_of_blk[bb] = ps_agg.tile(
                                [P, MSGW], f32, tag="agg", name=f"agg{l}_{bb}")
                            # self-loop contribution straight from the local
                            # table (opens the PSUM accumulation group)
                            lgs = sb_w.tile([P, H], f32, tag="lgs")
                            nc.vector.tensor_tensor(
                                out=lgs[:], in0=hsb[:, bb, 0:H],
                                in1=hsb[:, bb, H:2 * H], op=mybir.AluOpType.add)
                            lrs = sb_w.tile([P, H], f32, tag="lrs")
                            nc.vector.scalar_tensor_tensor(
                                out=lrs[:], in0=lgs[:], scalar=NEG_SLOPE,
                                in1=lgs[:], op0=mybir.AluOpType.mult,
                                op1=mybir.AluOpType.max)
                            exs = sb_w.tile([P, H, 1], bf16, tag="exs")
                            nc.scalar.activation(
                                exs[:].rearrange("p h o -> p (h o)"), lrs[:],
                                mybir.ActivationFunctionType.Exp)
                            msgs = sb_w.tile([P, MSGW], bf16, tag="msgs")
                            nc.vector.tensor_tensor(
                                out=msgs[:, 0:HC].rearrange("p (h c) -> p h c", h=H),
                                in0=hsb[:, bb, 2 * H:2 * H + HC]
                                    .rearrange("p (h c) -> p h c", h=H),
                                in1=exs[:].to_broadcast([P, H, Ch]),
                                op=mybir.AluOpType.mult)
                            nc.vector.tensor_copy(
                                out=msgs[:, HC:MSGW],
                                in_=exs[:].rearrange("p h o -> p (h o)"))
                            nc.tensor.matmul(agg_of_blk[bb][:], lhsT=identb[:],
                                             rhs=msgs[:], start=True, stop=False)
                        nc.tensor.matmul(agg_of_blk[bb][:], lhsT=sel4[:, j, :],
                                         rhs=msg4[:, j, :],
                                         start=False, stop=bool(sp))
                        if sp:
                            agg = agg_of_blk.pop(bb)
                            rec = sb_w.tile([P, H, 1], f32, tag="rec")
                            nc.vector.reciprocal(
                                rec[:].rearrange("p h o -> p (h o)"),
                                agg[:, HC:MSGW])
                            xb = sb_w.tile([P, HC], f32, tag="xb")
                            nc.vector.tensor_tensor(
                                out=xb[:].rearrange("p (h c) -> p h c", h=H),
                                in0=agg[:, 0:HC].rearrange("p (h c) -> p h c", h=H),
                                in1=rec[:].to_broadcast([P, H, Ch]),
                                op=mybir.AluOpType.mult)
                            nc.vector.tensor_scalar_max(xb[:], xb[:], 0.0)
                            if l < 2:
                                xps = ps_xp.tile([P, P], f32, tag="xps")
                                nc.tensor.transpose(xps[:], xb[:], ident[:])
                                nc.vector.tensor_copy(
                                    out=xT[:, bb * P:(bb + 1) * P], in_=xps[:])
                            else:
                                bsel = sb_w.tile([P, GRAPHS], f32, tag="bsel")
                                nc.vector.tensor_tensor(
                                    out=bsel[:],
                                    in0=batchf[:, bb, :].to_broadcast([P, GRAPHS]),
                                    in1=iota[:, 0, :GRAPHS],
                                    op=mybir.AluOpType.is_equal)
                                nc.tensor.matmul(pooled_ps[:], lhsT=bsel[:],
                                                 rhs=xb[:], start=(bb == 0),
                                                 stop=(bb == nb - 1))

        # ===== head =====
        pooled_sb = sb.tile([GRAPHS, HC], f32)
        nc.vector.tensor_copy(out=pooled_sb[:], in_=pooled_ps[:])
        pT_ps = ps_xp.tile([P, GRAPHS], f32, tag="xps")
        nc.tensor.transpose(pT_ps[:], pooled_sb[:], ident[:GRAPHS, :GRAPHS])
        pT_sb = sb.tile([P, GRAPHS], f32)
        nc.vector.tensor_copy(out=pT_sb[:], in_=pT_ps[:])
        log_ps = ps_xp.tile([GRAPHS, OUT], f32, tag="logps")
        nc.tensor.matmul(log_ps[:], lhsT=pT_sb[:], rhs=wh[:], start=True, stop=True)
        log_sb = sb.tile([GRAPHS, OUT], f32)
        nc.vector.tensor_copy(out=log_sb[:], in_=log_ps[:])
        nc.sync.dma_start(out=out_d[:], in_=log_sb[:])

    _fixup_wait_limits(nc)
    return nc


def prepare(x, Ws, a_srcs, a_dsts, biases, Wh, bh, edge_index, batch):
    n = x.shape[0]
    npad = int(math.ceil(n / (NCORES * P)) * NCORES * P)
    per = npad // NCORES
    nb = per // P

    x = np.asarray(x, np.float32)
    Ws = [np.asarray(w, np.float32) for w in Ws]
    a_srcs = [np.asarray(a, np.float32) for a in a_srcs]
    a_dsts = [np.asarray(a, np.float32) for a in a_dsts]
    Wh = np.asarray(Wh, np.float32)
    bh = np.asarray(bh, np.float32)
    edge_index = np.asarray(edge_index)
    batch = np.asarray(batch)
    for b in biases:
        assert np.allclose(np.asarray(b), 0.0), "nonzero GAT biases unsupported"

    # W_aug = [W@Ad | W@As | W]  (row layout [a_d | a_s | h])
    waugs = []
    for l in range(3):
        As = np.zeros((HC, H), np.float32)
        Ad = np.zeros((HC, H), np.float32)
        for h in range(H):
            As[h * Ch:(h + 1) * Ch, h] = a_srcs[l][h]
            Ad[h * Ch:(h + 1) * Ch, h] = a_dsts[l][h]
        W = Ws[l]
        waugs.append(np.concatenate([W @ Ad, W @ As, W], axis=1))
    waug = np.stack(waugs, 0)  # [3, 128, AUG]

    # self loops are handled on-chip (identity matmul per block), so only
    # the real edges go through the gather path
    src_all = edge_index[0].astype(np.int64)
    dst_all = edge_index[1].astype(np.int64)
    plan = _prep_edges(src_all, dst_all, per, nb)

    xpad = np.zeros((npad, HC), np.float32)
    xpad[:n] = x
    iota = np.tile(np.arange(P, dtype=np.float32)[None, :], (P, 4)).astype(BF16)
    identb = np.eye(P, dtype=np.float32).astype(BF16)

    batchf_full = np.full(npad, -1.0, np.float32)
    batchf_full[:n] = batch.astype(np.float32)

    nc = _build(npad, plan)

    in_maps = []
    for c in range(NCORES):
        sl = slice(c * per, (c + 1) * per)
        in_maps.append({
            "xT": np.ascontiguousarray(xpad[sl].T),
            "waug": waug,
            "wh": Wh,
            "iota": iota,
            "identb": identb,
            "srci": plan.srcs[c],
            "dloc": plan.dlocs[c],
            "batchf": np.ascontiguousarray(
                batchf_full[sl].reshape(nb, P).T).astype(BF16),
            })
    return nc, in_maps


def run_gat(x, Ws, a_srcs, a_dsts, biases, Wh, bh, edge_index, batch):
    nc, in_maps = prepare(x, Ws, a_srcs, a_dsts, biases, Wh, bh,
                          edge_index, batch)
    res = run_bass_kernel_spmd(nc, in_maps, list(range(NCORES)))
    global LAST_EXEC_NS
    LAST_EXEC_NS = getattr(res, "exec_time_ns", None)
    logits = np.zeros((GRAPHS, OUT), np.float32)
    for c in range(NCORES):
        logits += res.results[c]["out"]
    return logits + bh


def kernel(**inputs):
    return np.asarray(run_gat(
        inputs["x"], inputs["Ws"], inputs["a_srcs"], inputs["a_dsts"],
        inputs["biases"], inputs["Wh"], inputs["bh"], inputs["edge_index"],
        inputs["batch"]), np.float32)
